# revision 1
# baseline (speedup 1.0000x reference)
"""Trainium2 Bass kernel for the CPA block (sparse/efficient attention).

Strategy
--------
Data parallel over batch: B=128 -> 16 batch elements per NeuronCore, all
parameters replicated (folded on host into a handful of small matrices).

Per batch element (all shapes per core):
  pass 1 (ACT table set: natural_log_exp_and_others):
    - f2/f3/f4 load tokens-first, LayerNorm stats via bn_stats/bn_aggr,
      rstd = exp(-0.5*ln(var+eps)), normalize via fused tensor_scalar.
    - PE-transpose normalized activations to channels-first (bf16).
    - q1/q2 = Wq.T @ z2T (+ per-token bias via identity-matmul), exp on ACT
      with free running-sum (accum_out) giving the softmax denominators.
    - k/v tokens-first (z3T/z4T stationary), exp(k), per-head sums; the
      k-softmax denominator is folded into v, the q-softmax denominator is
      folded into the per-head context matrix.
    - ctx Gram matmuls (transposed), masked to block-diagonal, multiplied by
      rp_w on-chip -> M32/M42 [128,128]; attn = M32.T@eq1 + M42.T@eq2.
    - x = f2 + attn^T (PE transpose + fused add); LN4 stats; zx^T persisted.
  pass 2 (ACT table set: gelu_and_others):
    - ff1 -> Gelu (bias via ACT bias port) -> ff2 -> h^T -> out = x + h^T.

All matmuls run in bf16 (fp32 PSUM accumulate); the residual stream stays
fp32.  LayerNorm affine params and positional encodings are folded on the
host into the projection weights / per-token bias tiles.
"""

import os

import ml_dtypes
import numpy as np

NB = 16  # batch elements per core
NCORES = 8
EPS = 1e-5
N2, N3, N4, D, MLP = 1024, 256, 64, 128, 512
T2, T3 = N2 // 128, N3 // 128

_PROGRAM = None
LAST_RESULTS = None


def _build_program(nb=NB):
    from contextlib import ExitStack

    import concourse.bacc as bacc
    import concourse.mybir as mybir
    import concourse.tile as tile

    f32 = mybir.dt.float32
    bf16 = mybir.dt.bfloat16
    A = mybir.ActivationFunctionType
    Alu = mybir.AluOpType
    X = mybir.AxisListType.X

    # Restrict the activation-table menu so the load-insertion pass picks the
    # combined natural_log+exp set (serves Ln, Exp, Copy in pass 1) and the
    # gelu set (Gelu, Copy in pass 2) instead of thrashing between the
    # single-function sets. List indices stay aligned with act_info.json.
    class _Bacc(bacc.Bacc):
        _ACT_SETS = {"natural_log_exp_and_others", "gelu_and_others"}

        def insert_act_table_loads(self):
            import bass_rust as _bass_rust

            from concourse.hw_specs import get_activation_tables

            has_activation = any(
                isinstance(i, mybir.InstActivation)
                for b in self.main_func.blocks
                for i in b.instructions
            )
            if not has_activation:
                return
            tables = [
                (name, (fns if name in self._ACT_SETS else set()))
                for name, fns in get_activation_tables(self.m.arch).items()
            ]
            _bass_rust.insert_act_table_loads(self, tables)

    nc = _Bacc("TRN2", target_bir_lowering=False, debug=False)

    def din(name, shape, dt=f32):
        return nc.dram_tensor(name, shape, dt, kind="ExternalInput").ap()

    f2d = din("f2", [nb, N2, D])
    f3d = din("f3", [nb, N3, D])
    f4d = din("f4", [nb, N4, D])
    wq1d = din("wq1", [D, D], bf16)
    wq2d = din("wq2", [D, D], bf16)
    wk1d = din("wk1", [D, D], bf16)
    wv1d = din("wv1", [D, D], bf16)
    wk2d = din("wk2", [D, D], bf16)
    wv2d = din("wv2", [D, D], bf16)
    wrpd = din("wrp", [2, D, D], bf16)
    wff1d = din("wff1", [D, MLP], bf16)
    wff2d = din("wff2", [4, D, D], bf16)
    bq1td = din("bq1t", [D, N2], bf16)
    bq2td = din("bq2t", [D, N2], bf16)
    bk3d = din("bk3", [N3, D], bf16)
    bk4d = din("bk4", [N4, D], bf16)
    bv3d = din("bv3", [128, D], bf16)
    bv4d = din("bv4", [128, D], bf16)
    rpbd = din("rpb", [D, 1])
    ff1bd = din("ff1b", [D, 4])
    ff2bd = din("ff2b", [D, 1])
    identd = din("ident", [128, 128], bf16)
    outd = nc.dram_tensor("out", [nb, N2, D], f32, kind="ExternalOutput").ap()

    with tile.TileContext(nc) as tc, ExitStack() as ctx:
        consts = ctx.enter_context(tc.tile_pool(name="consts", bufs=1))
        state = ctx.enter_context(tc.tile_pool(name="state", bufs=1))
        work = ctx.enter_context(tc.tile_pool(name="work", bufs=2))
        small = ctx.enter_context(tc.tile_pool(name="small", bufs=2))
        pst = ctx.enter_context(tc.tile_pool(name="pst", bufs=2, space="PSUM"))
        psmm = ctx.enter_context(tc.tile_pool(name="psmm", bufs=2, space="PSUM"))
        psacc = ctx.enter_context(tc.tile_pool(name="psacc", bufs=2, space="PSUM"))
        pssm = ctx.enter_context(tc.tile_pool(name="pssm", bufs=2, space="PSUM"))

        def cload(name, shape, dt, src):
            t = consts.tile(shape, dt, name=name)
            nc.sync.dma_start(t, src)
            return t

        wq1 = cload("wq1_sb", [D, D], bf16, wq1d)
        wq2 = cload("wq2_sb", [D, D], bf16, wq2d)
        wk1 = cload("wk1_sb", [D, D], bf16, wk1d)
        wv1 = cload("wv1_sb", [D, D], bf16, wv1d)
        wk2 = cload("wk2_sb", [D, D], bf16, wk2d)
        wv2 = cload("wv2_sb", [D, D], bf16, wv2d)
        wrp0 = cload("wrp0_sb", [D, D], bf16, wrpd[0])
        wrp1 = cload("wrp1_sb", [D, D], bf16, wrpd[1])
        wff1 = cload("wff1_sb", [D, MLP], bf16, wff1d)
        wff2 = consts.tile([128, 4, 128], bf16, name="wff2_sb")
        nc.sync.dma_start(wff2, wff2d.rearrange("j k m -> k j m"))
        bq1t = cload("bq1t_sb", [D, N2], bf16, bq1td)
        bq2t = cload("bq2t_sb", [D, N2], bf16, bq2td)
        bk3 = consts.tile([128, T3, 128], bf16, name="bk3_sb")
        nc.sync.dma_start(bk3, bk3d.rearrange("(t p) d -> p t d", p=128))
        bk4 = cload("bk4_sb", [N4, D], bf16, bk4d)
        bv3 = cload("bv3_sb", [128, D], bf16, bv3d)
        bv4 = cload("bv4_sb", [128, D], bf16, bv4d)
        rpb = cload("rpb_sb", [D, 1], f32, rpbd)
        ff1b = cload("ff1b_sb", [D, 4], f32, ff1bd)
        ff2b = cload("ff2b_sb", [D, 1], f32, ff2bd)
        ident = cload("ident_sb", [128, 128], bf16, identd)

        eps_c = consts.tile([128, 1], f32, name="eps_c")
        nc.vector.memset(eps_c, EPS)
        x_all = state.tile([128, nb, T2, 128], f32, name="x_all")
        zxT_all = state.tile([128, nb * N2], bf16, name="zxT_all")

        def layernorm(src, ntiles, npart, tag):
            """src: [npart, ntiles, 128] f32 tokens-first -> z bf16 like src."""
            st = small.tile([npart, ntiles, 6], f32, tag=f"st_{tag}", name="st")
            for t in range(ntiles):
                nc.vector.bn_stats(st[:, t, :], src[:, t, :])
            mv = small.tile([npart, ntiles, 2], f32, tag=f"mv_{tag}", name="mv")
            for t in range(ntiles):
                nc.vector.bn_aggr(mv[:, t, :], st[:, t, :])
            rstd = small.tile([npart, ntiles], f32, tag=f"rstd_{tag}", name="rstd")
            nc.vector.tensor_scalar(rstd, mv[:, :, 1], EPS, None, Alu.add)
            nc.scalar.activation(rstd, rstd, A.Ln)
            nc.scalar.activation(rstd, rstd, A.Exp, scale=-0.5)
            z = work.tile([npart, ntiles, 128], bf16, tag=f"z_{tag}", name="z")
            for t in range(ntiles):
                nc.vector.tensor_scalar(
                    z[:, t, :], src[:, t, :], mv[:, t, 0:1], rstd[:, t : t + 1],
                    Alu.subtract, Alu.mult,
                )
            return z

        def transpose_to(dst, z, ntiles, use_act):
            """z: [128, ntiles, 128] bf16 -> dst [128, ntiles*128] bf16 (chan-first)."""
            for i in range(0, ntiles, 4):
                j = min(i + 4, ntiles)
                ps = pst.tile([128, 512], bf16, tag="pst", name="ps")
                for q in range(i, j):
                    nc.tensor.transpose(
                        ps[:, (q - i) * 128 : (q - i + 1) * 128], z[:, q, :], ident
                    )
                if use_act:
                    nc.scalar.activation(
                        dst[:, i * 128 : j * 128], ps[:, : (j - i) * 128], A.Copy
                    )
                else:
                    nc.vector.tensor_copy(dst[:, i * 128 : j * 128], ps[:, : (j - i) * 128])

        # ---------------- pass 1 ----------------
        for b in range(nb):
            f2t = work.tile([128, T2, 128], f32, tag="f2t", name="f2t")
            nc.sync.dma_start(f2t, f2d[b].rearrange("(t p) d -> p t d", p=128))
            f3t = work.tile([128, T3, 128], f32, tag="f3t", name="f3t")
            nc.sync.dma_start(f3t, f3d[b].rearrange("(t p) d -> p t d", p=128))
            f4t = work.tile([N4, 1, 128], f32, tag="f4t", name="f4t")
            nc.sync.dma_start(f4t[:, 0, :], f4d[b])

            z2 = layernorm(f2t, T2, 128, "t2")
            z3 = layernorm(f3t, T3, 128, "t3")
            z4 = layernorm(f4t, 1, N4, "t4")

            z2T = work.tile([128, N2], bf16, tag="z2T", name="z2T")
            transpose_to(z2T, z2, T2, use_act=True)
            z3T = work.tile([128, N3], bf16, tag="z3T", name="z3T")
            transpose_to(z3T, z3, T3, use_act=True)
            z4T = work.tile([128, N4], bf16, tag="z4T", name="z4T")
            ps4 = pst.tile([128, 512], bf16, tag="pst", name="ps4")
            nc.tensor.transpose(ps4[:, :N4], z4[:, 0, :], ident[:N4, :N4])
            nc.scalar.activation(z4T, ps4[:, :N4], A.Copy)

            # q projections + exp (+ per-channel running sums)
            eq1 = work.tile([128, N2], bf16, tag="eq1", name="eq1")
            eq2 = work.tile([128, N2], bf16, tag="eq2", name="eq2")
            S = small.tile([128, 2, 2], f32, tag="S", name="S")  # [q12, chunk]
            for qi, (wq, bqt, eq) in enumerate(((wq1, bq1t, eq1), (wq2, bq2t, eq2))):
                for c in range(2):
                    qp = psmm.tile([128, 512], f32, tag="mm", name="qp")
                    nc.tensor.matmul(qp, wq, z2T[:, c * 512 : (c + 1) * 512],
                                     start=True, stop=False)
                    nc.tensor.matmul(qp, ident, bqt[:, c * 512 : (c + 1) * 512],
                                     start=False, stop=True)
                    nc.scalar.activation(eq[:, c * 512 : (c + 1) * 512], qp, A.Exp,
                                         accum_out=S[:, qi, c : c + 1])
            rS = small.tile([128, 2], f32, tag="rS", name="rS")
            nc.vector.tensor_tensor(rS, S[:, :, 0], S[:, :, 1], Alu.add)
            nc.vector.reciprocal(rS, rS)

            # k3/v3 (tokens-first; z3T tiles stationary)
            kv3p = psmm.tile([128, 512], f32, tag="mm", name="kv3p")
            for t in range(T3):
                z3s = z3T[:, t * 128 : (t + 1) * 128]
                nc.tensor.matmul(kv3p[:, t * 128 : (t + 1) * 128], z3s, wk1,
                                 start=True, stop=True)
                nc.tensor.matmul(kv3p[:, 256 + t * 128 : 256 + (t + 1) * 128], z3s,
                                 wv1, start=True, stop=True)
            k3f = work.tile([128, T3, 128], f32, tag="k3f", name="k3f")
            nc.vector.tensor_tensor(
                k3f, kv3p[:, 0:256].rearrange("p (t d) -> p t d", t=T3), bk3, Alu.add
            )
            ek3 = work.tile([128, T3, 128], bf16, tag="ek3", name="ek3")
            nc.scalar.activation(ek3, k3f, A.Exp)
            s3 = small.tile([128, T3, 2], f32, tag="s3", name="s3")
            nc.vector.tensor_reduce(
                s3, ek3.rearrange("p t (h e) -> p t h e", h=2), axis=X, op=Alu.add
            )
            nc.vector.reciprocal(s3, s3)
            v3s = work.tile([128, T3, 128], bf16, tag="v3s", name="v3s")
            for t in range(T3):
                vtile = small.tile([128, 128], f32, tag="vtile", name="vtile")
                nc.vector.tensor_tensor(
                    vtile, kv3p[:, 256 + t * 128 : 256 + (t + 1) * 128], bv3, Alu.add
                )
                for h in range(2):
                    nc.vector.tensor_scalar(
                        v3s[:, t, h * 64 : (h + 1) * 64],
                        vtile[:, h * 64 : (h + 1) * 64],
                        s3[:, t, h : h + 1], None, Alu.mult,
                    )

            # k4/v4
            kv4p = pssm.tile([N4, 512], f32, tag="sm", name="kv4p")
            nc.tensor.matmul(kv4p[:, 0:128], z4T, wk2, start=True, stop=True)
            nc.tensor.matmul(kv4p[:, 128:256], z4T, wv2, start=True, stop=True)
            k4f = work.tile([N4, 128], f32, tag="k4f", name="k4f")
            nc.vector.tensor_tensor(k4f, kv4p[:, 0:128], bk4, Alu.add)
            ek4 = work.tile([N4, 128], bf16, tag="ek4", name="ek4")
            nc.scalar.activation(ek4, k4f, A.Exp)
            s4 = small.tile([N4, 1, 2], f32, tag="s4", name="s4")
            nc.vector.tensor_reduce(
                s4, ek4.rearrange("p (o h e) -> p o h e", o=1, h=2), axis=X, op=Alu.add
            )
            nc.vector.reciprocal(s4, s4)
            v4s = work.tile([N4, 128], bf16, tag="v4s", name="v4s")
            for h in range(2):
                vtile4 = small.tile([N4, 64], f32, tag="vtile4", name="vtile4")
                nc.vector.tensor_tensor(
                    vtile4, kv4p[:, 128 + h * 64 : 128 + (h + 1) * 64],
                    bv4[:N4, h * 64 : (h + 1) * 64], Alu.add
                )
                nc.vector.tensor_scalar(
                    v4s[:, h * 64 : (h + 1) * 64], vtile4, s4[:, 0, h : h + 1],
                    None, Alu.mult,
                )

            # Gram (transposed): g[e, d] = sum_tok v[tok,e] k[tok,d]
            g32p = pssm.tile([128, 128], f32, tag="sm", name="g32p")
            for t in range(T3):
                nc.tensor.matmul(g32p, v3s[:, t, :], ek3[:, t, :],
                                 start=(t == 0), stop=(t == T3 - 1))
            g42p = pssm.tile([128, 128], f32, tag="sm", name="g42p")
            nc.tensor.matmul(g42p, v4s, ek4, start=True, stop=True)

            # mask to block-diag, fold rp_w and 1/S -> M matrices
            ms = []
            for gi, (gp, wrp_, qi) in enumerate(((g32p, wrp0, 0), (g42p, wrp1, 1))):
                gm = work.tile([128, 128], bf16, tag="gm", name="gm")
                nc.vector.memset(gm, 0)
                for h in range(2):
                    nc.vector.tensor_copy(
                        gm[h * 64 : (h + 1) * 64, h * 64 : (h + 1) * 64],
                        gp[h * 64 : (h + 1) * 64, h * 64 : (h + 1) * 64],
                    )
                mp = pssm.tile([128, 128], f32, tag="sm", name="mp")
                nc.tensor.matmul(mp, gm, wrp_, start=True, stop=True)
                m = work.tile([128, 128], bf16, tag="m_", name="m_")
                nc.vector.tensor_scalar(m, mp, rS[:, qi : qi + 1], None, Alu.mult)
                ms.append(m)
            m32, m42 = ms

            # attn = M32.T @ eq1 + M42.T @ eq2  (channels-first), + rp_b
            attn = work.tile([128, N2], bf16, tag="attn", name="attn")
            for c in range(2):
                ap_ = psmm.tile([128, 512], f32, tag="mm", name="ap_")
                nc.tensor.matmul(ap_, m32, eq1[:, c * 512 : (c + 1) * 512],
                                 start=True, stop=False)
                nc.tensor.matmul(ap_, m42, eq2[:, c * 512 : (c + 1) * 512],
                                 start=False, stop=True)
                nc.vector.tensor_scalar(attn[:, c * 512 : (c + 1) * 512], ap_, rpb,
                                        None, Alu.add)

            # x = f2 + attn^T
            x_b = x_all[:, b]
            for half in range(2):
                ps = pst.tile([128, 512], bf16, tag="pst", name="psx")
                for q in range(4):
                    t = half * 4 + q
                    nc.tensor.transpose(
                        ps[:, q * 128 : (q + 1) * 128],
                        attn[:, t * 128 : (t + 1) * 128], ident,
                    )
                nc.vector.tensor_tensor(
                    x_b[:, half * 4 : (half + 1) * 4, :].rearrange("p t d -> p (t d)"),
                    f2t[:, half * 4 : (half + 1) * 4, :].rearrange("p t d -> p (t d)"),
                    ps, Alu.add,
                )

            zx = layernorm(x_b, T2, 128, "t2x")
            transpose_to(zxT_all[:, b * N2 : (b + 1) * N2], zx, T2, use_act=False)

        # ---------------- pass 2 ----------------
        for b in range(nb):
            zxT = zxT_all[:, b * N2 : (b + 1) * N2]
            hcf = work.tile([128, N2], bf16, tag="hcf", name="hcf")
            for c in range(2):
                hp = psacc.tile([128, 512], f32, tag="acc", name="hp")
                for j in range(4):
                    gp = psmm.tile([128, 512], f32, tag="mm", name="gp")
                    nc.tensor.matmul(gp, wff1[:, j * 128 : (j + 1) * 128],
                                     zxT[:, c * 512 : (c + 1) * 512],
                                     start=True, stop=True)
                    gj = work.tile([128, 512], bf16, tag="gj", name="gj")
                    nc.scalar.activation(gj, gp, A.Gelu, bias=ff1b[:, j : j + 1])
                    nc.tensor.matmul(hp, wff2[:, j, :], gj,
                                     start=(j == 0), stop=(j == 3),
                                     skip_group_check=True)
                nc.vector.tensor_scalar(hcf[:, c * 512 : (c + 1) * 512], hp, ff2b,
                                        None, Alu.add)
            x_b = x_all[:, b]
            for half in range(2):
                ps = pst.tile([128, 512], bf16, tag="pst", name="psh")
                for q in range(4):
                    t = half * 4 + q
                    nc.tensor.transpose(
                        ps[:, q * 128 : (q + 1) * 128],
                        hcf[:, t * 128 : (t + 1) * 128], ident,
                    )
                xv = x_b[:, half * 4 : (half + 1) * 4, :].rearrange("p t d -> p (t d)")
                nc.vector.tensor_tensor(xv, xv, ps, Alu.add)
            nc.sync.dma_start(outd[b].rearrange("(t p) d -> p t d", p=128), x_all[:, b])

    nc.compile()
    return nc


def _get_program():
    global _PROGRAM
    if _PROGRAM is None:
        _PROGRAM = _build_program(NB)
    return _PROGRAM


def _prepare_params(inputs):
    bf = ml_dtypes.bfloat16
    g = {k: np.asarray(v, np.float32) for k, v in inputs.items()
         if k not in ("f2", "f3", "f4")}
    pe2, pe3, pe4 = g["pe2"][0], g["pe3"][0], g["pe4"][0]

    def fold_w(ln_w, w):
        return np.ascontiguousarray(ln_w[:, None] * w).astype(bf)

    def fold_bt(ln_b, pe, w, b):
        return np.ascontiguousarray(((ln_b[None, :] + pe) @ w + b[None, :]).T).astype(bf)

    p = {}
    p["wq1"] = fold_w(g["ln1_w"], g["q1_w"])
    p["wq2"] = fold_w(g["ln1_w"], g["q2_w"])
    p["wk1"] = fold_w(g["ln2_w"], g["k1_w"])
    p["wv1"] = fold_w(g["ln2_w"], g["v1_w"])
    p["wk2"] = fold_w(g["ln3_w"], g["k2_w"])
    p["wv2"] = fold_w(g["ln3_w"], g["v2_w"])
    p["bq1t"] = fold_bt(g["ln1_b"], pe2, g["q1_w"], g["q1_b"])
    p["bq2t"] = fold_bt(g["ln1_b"], pe2, g["q2_w"], g["q2_b"])
    p["bk3"] = np.ascontiguousarray(
        (g["ln2_b"][None, :] + pe3) @ g["k1_w"] + g["k1_b"][None, :]).astype(bf)
    p["bk4"] = np.ascontiguousarray(
        (g["ln3_b"][None, :] + pe4) @ g["k2_w"] + g["k2_b"][None, :]).astype(bf)
    bv3row = g["ln2_b"] @ g["v1_w"] + g["v1_b"]
    bv4row = g["ln3_b"] @ g["v2_w"] + g["v2_b"]
    p["bv3"] = np.ascontiguousarray(np.tile(bv3row[None, :], (128, 1))).astype(bf)
    p["bv4"] = np.ascontiguousarray(np.tile(bv4row[None, :], (128, 1))).astype(bf)
    p["wrp"] = np.ascontiguousarray(
        g["rp_w"].reshape(2, D, D)).astype(bf)
    p["rpb"] = np.ascontiguousarray(g["rp_b"][:, None]).astype(np.float32)
    p["wff1"] = fold_w(g["ln4_w"], g["ff1_w"])
    bff1 = g["ln4_b"] @ g["ff1_w"] + g["ff1_b"]
    p["ff1b"] = np.ascontiguousarray(bff1.reshape(4, 128).T).astype(np.float32)
    p["wff2"] = np.ascontiguousarray(g["ff2_w"].reshape(4, 128, D)).astype(bf)
    p["ff2b"] = np.ascontiguousarray(g["ff2_b"][:, None]).astype(np.float32)
    p["ident"] = np.eye(128, dtype=np.float32).astype(bf)
    return p


def kernel(**inputs):
    global LAST_RESULTS
    from concourse import bass_utils

    f2 = np.ascontiguousarray(np.asarray(inputs["f2"], np.float32))
    f3 = np.ascontiguousarray(np.asarray(inputs["f3"], np.float32))
    f4 = np.ascontiguousarray(np.asarray(inputs["f4"], np.float32))
    params = _prepare_params(inputs)
    nc = _get_program()

    in_maps = []
    for c in range(NCORES):
        m = dict(params)
        sl = slice(c * NB, (c + 1) * NB)
        m["f2"] = f2[sl]
        m["f3"] = f3[sl]
        m["f4"] = f4[sl]
        in_maps.append(m)

    res = bass_utils.run_bass_kernel_spmd(
        nc, in_maps, list(range(NCORES)),
        trace=bool(int(os.environ.get("KERNEL_TRACE", "0"))),
    )
    LAST_RESULTS = res
    out = np.concatenate([r["out"] for r in res.results], axis=0)
    return np.ascontiguousarray(out.astype(np.float32))

